# revision 25
# baseline (speedup 1.0000x reference)
"""Trainium2 Bass kernel for ClassAttentionTSSA.

Reference computation (B=64, C=256, T=64, V=25, h=8, hd=32):
    xc = x_cls  as (B, V, C) tokens;  xp = x_patch as (B, T*V, C) tokens
    q = xc @ q_w.T ; k = xp @ k_w.T ; v = xp @ v_w.T   (per-head split hd=32)
    S = (q @ k.T) * scale * temp_h ; A = softmax(S) ; o = A @ v
    y = concat_heads(o) @ proj_w.T + proj_b  -> (B, C, 1, V)

Weight-only reassociations (exact up to fp reordering):
    S_h = xc @ G_h @ xp.T    with G_h = (q_w*scale*temp)_h.T @ k_w_h  (C x C)
    y   = sum_h (A_h @ xp) @ W_h.T + b   with W_h = proj_w[:,h] @ v_w[h,:]
so q/k/v are never materialized.  Channels live on partitions, tokens on
the free dim.  All inputs are DMA'd into SBUF ONCE at program start
(everything fits; x_patch is shipped in both layouts: [cin,kt] as fp8e4
for the S matmuls and [kt,cin] as bf16 for the ctx matmuls), so the
steady-state rep loop is pure compute + one output DMA.

Per-core pipeline per rep (8 batches, software-pipelined per step s:
recip(s-1) | S+exp(s) | ctx(s-1) | Z(s)):
    B) S^T chunk-group matmuls: ONE fp8e4 DoubleRow matmul per 128-kt
       chunk (K=256 contracted in one instruction at 0.5 cyc/col).
       qkT is pre-scaled by ALPHA=128 (folded into G on the host) so
       fp8e4 sees ~unit-variance values; the Exp's free scale undoes it.
       Softmax numerator: ONE ACT Exp per 4-chunk PSUM group, straight
       PSUM->SBUF bf16.  No max-subtraction (|S| < 1 for this data).
    C) Z: the 12 full attn chunks fold pairwise on DVE (bf16 level 1,
       f32 above -- one rounding total), dependent only on Exps 1-3;
       then TWO all-ones matmuls on PE (folded sum + chunk 12) that
       partition-reduce AND broadcast Z into PSUM in one accumulation
       group, then DVE reciprocal_approx_fast.
    D) ctxT accumulation matmuls (bf16), normalized by 1/Z on DVE.
    E) y^T = sum_h W_h^T @ ctxT + pb, DMA out.

Accuracy: rel-to-absmax ~9.6e-3 (gate 2e-2); the fp8 S-side is the
dominant term (bf16-everything measures ~3.4e-3); validated in numpy
before committing.  fp8 on the ctx side fails (4.1e-2) -- do not.

Perf notes for this axon-tunneled environment (all MEASURED here):
  - HW exec: 48.3us/rep (baseline inherited: 158.8us; same-code
    run-to-run drift up to ~15% was observed across the session).
    PE is instruction-ISSUE-limited (~110ns/matmul): per batch
    13 S + 2 Z + 26 ctx matmuls + 32 for stage E per rep.
  - For_i back-edge all-engine barrier ~4.1us/iter.  Putting MULTIPLE
    rep bodies inside one For_i iteration REGRESSES badly (+19us per
    extra body: 2-body 69us/rep, 4-body 80us/rep) -- do not unroll.
    For_i(staggered_reset=True) hard-crashes the device (NRT 101).
  - ACT table funcs cost the same as Copy (~670ns/800col + ~390ns
    fixed): Exp==Square==Copy.  The previous session's "Exp ~28us"
    note was wrong; exp-via-DVE-quartic is unnecessary.
  - DVE: TS bf16 2x (~428ns/800col), TT bf16 2x (~363ns/800col), STT
    and TensorReduce have NO fast modes (1x); reciprocal is 8cyc/elem
    (use reciprocal_approx_fast, ~5x faster, 18-bit); DVE ops may read
    at most ONE non-scalar input from PSUM.
  - Cross-engine semaphore latency is small (~0.1-0.2us), but work
    placed AFTER the last Exp of a batch lands on the per-step
    critical path: keep the post-Exp4 tail to ONE Z matmul (chunk 12);
    folds that depend on Exp4 cost ~+1us/step (measured 59us).
  - gpsimd compute ops ~16us each (avoid; SWDGE cast-DMA is fine).

Sharding: data-parallel over batch, 8 batches per NeuronCore, 8 cores.
"""

import math
import sys

sys.path.insert(0, "/opt/trn_rl_repo")

import numpy as np
import ml_dtypes

import concourse.bacc as bacc
import concourse.mybir as mybir
import concourse.tile as tile
from concourse import bass_utils

B, C, T, V = 64, 256, 64, 25
H, HD = 8, 32
KT = T * V            # 1600 key tokens
NCORES = 8
BLOC = B // NCORES    # 8 batches per core
R = H * V             # 200 packed (head, query) columns per batch
CK = C // 128         # 2 channel chunks

F32 = mybir.dt.float32
BF16 = mybir.dt.bfloat16
FP8 = mybir.dt.float8e4
ALPHA = 128.0         # qkT pre-scale so fp8e4 uses its dynamic range;
                      # folded into G on the host, undone by the Exp scale

KT_CHUNKS = [128] * (KT // 128) + ([KT % 128] if KT % 128 else [])
NM = len(KT_CHUNKS)   # 13
NM_FULL = KT // 128   # 12
TILE_CH = 4           # S^T chunks per 2-bank psum tile (2 per bank)

_PROG_CACHE = {}
_SIM_UNROLL = False   # sim tooling sets True: plain body instead of For_i
                      # (TimelineSim cannot resolve register-mode branches)


def _build_program(nreps: int = 1):
    """Build + compile the per-core Bass program (same program on all cores)."""
    from contextlib import ExitStack, nullcontext

    MULT, ADD = mybir.AluOpType.mult, mybir.AluOpType.add
    AF = mybir.ActivationFunctionType

    nc = bacc.Bacc("TRN2", target_bir_lowering=False, debug=False)

    xc_d = nc.dram_tensor("xc", [BLOC, C, V], F32, kind="ExternalInput")
    xpb_d = nc.dram_tensor("xpb", [BLOC, C, KT], FP8, kind="ExternalInput")
    xpt_d = nc.dram_tensor("xpt", [BLOC, KT, C], BF16, kind="ExternalInput")
    g_d = nc.dram_tensor("g", [H, C, C], BF16, kind="ExternalInput")
    w_d = nc.dram_tensor("w", [H, C, C], BF16, kind="ExternalInput")
    pb_d = nc.dram_tensor("pb", [C, 1], F32, kind="ExternalInput")
    y_d = nc.dram_tensor("y", [BLOC, C, V], F32, kind="ExternalOutput")

    with tile.TileContext(nc) as tc, ExitStack() as es:
        wpool = es.enter_context(tc.tile_pool(name="weights", bufs=1))
        xpool = es.enter_context(tc.tile_pool(name="xdata", bufs=1))
        attn_pool = es.enter_context(tc.tile_pool(name="attn", bufs=1))
        zpool = es.enter_context(tc.tile_pool(name="zdata", bufs=1))
        ysb_pool = es.enter_context(tc.tile_pool(name="ysb", bufs=2))

        # ---- persistent weights / activations (one DMA each) ----
        g_sb = wpool.tile([128, H * CK * C], BF16, tag="g")
        nc.sync.dma_start(
            g_sb[:].rearrange("p (h kc j) -> p h kc j", h=H, kc=CK),
            g_d.ap().rearrange("h (kc p) j -> p h kc j", kc=CK),
        )
        w_sb = wpool.tile([128, H * CK * C], BF16, tag="w")
        nc.sync.dma_start(
            w_sb[:].rearrange("p (h kc j) -> p h kc j", h=H, kc=CK),
            w_d.ap().rearrange("h (kc p) j -> p h kc j", kc=CK),
        )
        pb_sb = wpool.tile([128, CK], F32, tag="pb")
        nc.sync.dma_start(
            pb_sb[:], pb_d.ap().rearrange("(kc p) one -> p (kc one)", kc=CK))
        xcT = wpool.tile([128, CK * BLOC * V], BF16, tag="xc")
        for kc in range(CK):
            nc.gpsimd.dma_start(  # SWDGE: casts f32 -> bf16 in flight
                xcT[:, kc * BLOC * V:(kc + 1) * BLOC * V].rearrange(
                    "p (b v) -> p b v", b=BLOC),
                xc_d.ap()[:, kc * 128:(kc + 1) * 128, :].rearrange(
                    "b p v -> p b v"),
            )
        ones_sb = wpool.tile([128, 128], BF16, tag="ones")
        nc.vector.memset(ones_sb[:], 1.0)
        ones_f32 = wpool.tile([128, 128], F32, tag="ones32")
        nc.vector.memset(ones_f32[:], 1.0)
        zero_bias = wpool.tile([128, 1], F32, tag="zb")
        nc.vector.memset(zero_bias[:], 0.0)

        # qkT cols: (kc | b, h, qi)  b-major: S^T rhs slices contiguous
        qkT = wpool.tile([128, CK * BLOC * R], FP8, tag="qkT")
        # ctxT cols: (kc | h, b, qi) h-major: y rhs slices contiguous
        ctxT = wpool.tile([128, CK * BLOC * R], BF16, tag="ctxT")

        # per-batch persistent input tiles + attn tiles (loaded ONCE)
        xpb_sb = [xpool.tile([128, CK * KT], FP8, tag=f"xpb{b}",
                             name=f"xpb{b}") for b in range(BLOC)]
        xpt_sb = [xpool.tile([128, NM * C], BF16, tag=f"xpt{b}",
                             name=f"xpt{b}") for b in range(BLOC)]
        attn_sb = [attn_pool.tile([128, NM * R], BF16, tag=f"attn{b}",
                              name=f"attn{b}") for b in range(BLOC)]
        for b in range(BLOC):
            nc.sync.dma_start(
                xpb_sb[b][:].rearrange("p (kc j) -> p kc j", kc=CK),
                xpb_d.ap()[b].rearrange("(kc p) j -> p kc j", kc=CK),
            )
            nc.sync.dma_start(
                xpt_sb[b][:, 0:NM_FULL * C].rearrange(
                    "p (m j) -> p m j", m=NM_FULL),
                xpt_d.ap()[b, 0:NM_FULL * 128, :].rearrange(
                    "(m p) j -> p m j", p=128),
            )
            nc.sync.dma_start(
                xpt_sb[b][0:KT - NM_FULL * 128, NM_FULL * C:NM * C],
                xpt_d.ap()[b, NM_FULL * 128:KT, :],
            )
            # rows 64:128 of the 64-row last chunk are never written by
            # the exp -- zero them once so the Z-reduce sees zeros there.
            nc.vector.memset(attn_sb[b][KT % 128:128, NM_FULL * R:NM * R], 0.0)

        recip = zpool.tile([128, BLOC * R], F32, tag="recip")
        # pairwise folds of attn chunks 0-11 on DVE (bf16 level 1 only --
        # later levels are f32 so Z keeps one rounding); depends only on
        # Exps 1-3, so the post-Exp4 critical tail stays 1 matmul.
        zs_sb = [zpool.tile([128, 6 * R], BF16, tag=f"zs{b}",
                            name=f"zs{b}") for b in range(BLOC)]
        zs2_sb = [zpool.tile([128, 3 * R], F32, tag=f"zs2{b}",
                             name=f"zs2{b}") for b in range(BLOC)]
        za_sb = [zpool.tile([128, R], F32, tag=f"za{b}",
                            name=f"za{b}") for b in range(BLOC)]
        zsum_sb = [zpool.tile([128, R], F32, tag=f"zsum{b}",
                              name=f"zsum{b}") for b in range(BLOC)]

        # ---- phase 1: qkT[cin, (b,h,qi)] = G_h^T @ xcT ----
        with tc.tile_pool(name="ps_qk", bufs=2, space="PSUM") as ps_qk:
            for mc in range(CK):
                for hg in range(2):          # head groups of 4
                    pq = ps_qk.tile([128, 4 * 512], F32, tag="pq")
                    for i in range(4):
                        h = hg * 4 + i
                        for kc in range(CK):
                            nc.tensor.matmul(
                                pq[:, i * 512:i * 512 + BLOC * V],
                                g_sb[:, (h * CK + kc) * C + mc * 128:
                                     (h * CK + kc) * C + mc * 128 + 128],
                                xcT[:, kc * BLOC * V:(kc + 1) * BLOC * V],
                                start=(kc == 0), stop=(kc == CK - 1),
                            )
                    nc.vector.tensor_copy(
                        qkT[:, mc * BLOC * R:(mc + 1) * BLOC * R]
                        .rearrange("p (b h q) -> p b h q", b=BLOC, h=H)
                        [:, :, hg * 4:(hg + 1) * 4, :],
                        pq[:].rearrange("p (i n) -> p i n", i=4)
                        [:, :, 0:BLOC * V]
                        .rearrange("p i (b q) -> p b i q", q=V),
                    )

        ps_st = es.enter_context(
            tc.tile_pool(name="ps_st", bufs=2, space="PSUM"))
        ps_z = es.enter_context(
            tc.tile_pool(name="ps_z", bufs=2, space="PSUM"))
        ps_acc = es.enter_context(
            tc.tile_pool(name="ps_acc", bufs=2, space="PSUM"))

        with (nullcontext(0) if _SIM_UNROLL else tc.For_i(0, nreps)) as _rep:
            # ---- stages B/C/D: per-batch software pipeline ----
            # step s emits S^T+exp for batch s, then Z/recip/ctx for
            # batch s-1, so PE's ctx matmuls overlap the ACT-limited
            # stretch of the next batch's exp.
            pz_live = {}

            def emit_front(b):
                # S^T chunk-group matmuls + one ACT Exp per group
                m = 0
                while m < NM:
                    gsz = min(TILE_CH, NM - m)
                    if KT_CHUNKS[m + gsz - 1] != KT_CHUNKS[m]:
                        gsz -= 1
                    rows = KT_CHUNKS[m]
                    st = ps_st.tile([128, 2 * 512], F32, tag="st")
                    for i in range(gsz):
                        # fp8 DoubleRow: contracts both 128-channel halves
                        # (K=256) in ONE matmul at half cycles/col
                        nc.tensor.matmul(
                            st[0:rows, i * 256:i * 256 + R],
                            xpb_sb[b][:].rearrange(
                                "p (kc j) -> p kc j", kc=CK)
                            [:, :, (m + i) * 128:
                             (m + i) * 128 + KT_CHUNKS[m + i]],
                            qkT[:].rearrange(
                                "p (kc n) -> p kc n", kc=CK)
                            [:, :, b * R:(b + 1) * R],
                            start=True, stop=True,
                            perf_mode=mybir.MatmulPerfMode.DoubleRow,
                        )
                    x = st[0:rows, :].rearrange(
                        "p (g n) -> p g n", n=256)[:, 0:gsz, 0:R]
                    # softmax numerator straight from PSUM on ACT
                    nc.scalar.activation(
                        attn_sb[b][0:rows, m * R:(m + gsz) * R].rearrange(
                            "p (g n) -> p g n", g=gsz),
                        x, AF.Exp, bias=zero_bias[0:rows, :],
                        scale=1.0 / ALPHA)
                    m += gsz
                    if m == 12:
                        nc.vector.tensor_add(
                            zs_sb[b][:], attn_sb[b][:, 0:6 * R],
                            attn_sb[b][:, 6 * R:12 * R])
                        nc.vector.tensor_add(
                            zs2_sb[b][:], zs_sb[b][:, 0:3 * R],
                            zs_sb[b][:, 3 * R:6 * R])
                        nc.vector.tensor_add(
                            za_sb[b][:], zs2_sb[b][:, 0:R],
                            zs2_sb[b][:, R:2 * R])
                        nc.vector.tensor_add(
                            zsum_sb[b][:], za_sb[b][:],
                            zs2_sb[b][:, 2 * R:3 * R])

            def emit_pz(b):
                # Z on PE: one all-ones matmul per exp'd chunk (contracts
                # over kt partitions AND broadcasts over the 128 out
                # partitions in one op).  Emitted late in the step so the
                # Exp(b) -> pz(b) wait has slack behind ctx(b-1).
                pz = ps_z.tile([128, 512], F32, tag="pz")
                pz_live[b] = pz
                nc.tensor.matmul(
                    pz[:, 0:R], ones_f32[:], zsum_sb[b][:],
                    start=True, stop=False,
                )
                nc.tensor.matmul(
                    pz[:, 0:R], ones_sb[:],
                    attn_sb[b][:, 12 * R:13 * R],
                    start=False, stop=True,
                )

            def emit_recip(b):
                pz = pz_live.pop(b)
                nc.vector.reciprocal_approx_fast(
                    recip[:, b * R:(b + 1) * R], pz[:, 0:R])

            def emit_ctx(b):
                # ctxT[cin, (h,qi)] = sum_kt xp_kt^T @ A^T, * 1/Z
                for mc in range(CK):
                    pc = ps_acc.tile([128, 512], F32, tag="pc")
                    for m in range(NM):
                        nc.tensor.matmul(
                            pc[:, 0:R],
                            xpt_sb[b][0:KT_CHUNKS[m], m * C + mc * 128:
                                      m * C + mc * 128 + 128],
                            attn_sb[b][0:KT_CHUNKS[m], m * R:(m + 1) * R],
                            start=(m == 0), stop=(m == NM - 1),
                        )
                    nc.vector.tensor_mul(
                        ctxT[:, mc * BLOC * R:(mc + 1) * BLOC * R].rearrange(
                            "p (h b q) -> p h b q", h=H, b=BLOC)[:, :, b, :],
                        pc[:, 0:R].rearrange("p (h q) -> p h q", h=H),
                        recip[:, b * R:(b + 1) * R].rearrange(
                            "p (h q) -> p h q", h=H),
                    )

            def rep_body():
                for s in range(BLOC + 1):
                    if s >= 1:
                        emit_recip(s - 1)
                    if s < BLOC:
                        emit_front(s)
                    if s >= 1:
                        emit_ctx(s - 1)
                    if s < BLOC:
                        emit_pz(s)

                # ---- stage E: y^T = sum_h W_h^T @ ctxT + pb ----
                for mc in range(CK):
                    py = ps_acc.tile([128, 512], F32, tag="pc")
                    idx = 0
                    for h in range(H):
                        for kc in range(CK):
                            nc.tensor.matmul(
                                py[:, 0:BLOC * V],
                                w_sb[:, (h * CK + kc) * C + mc * 128:
                                     (h * CK + kc) * C + mc * 128 + 128],
                                ctxT[:, kc * BLOC * R + h * BLOC * V:
                                     kc * BLOC * R + (h + 1) * BLOC * V],
                                start=(idx == 0), stop=(idx == 2 * H - 1),
                            )
                            idx += 1
                    ysb = ysb_pool.tile([128, BLOC * V], F32, tag="ysb")
                    nc.vector.tensor_scalar_add(
                        ysb[:], py[:, 0:BLOC * V], pb_sb[:, mc:mc + 1])
                    nc.sync.dma_start(
                        y_d.ap()[:, mc * 128:(mc + 1) * 128, :].rearrange(
                            "b p v -> p b v"),
                        ysb[:].rearrange("p (b v) -> p b v", b=BLOC),
                    )

            if _SIM_UNROLL:
                for _ in range(nreps - 1):
                    rep_body()
            rep_body()

    nc.compile()
    return nc


def _get_program(nreps: int = 1):
    if nreps not in _PROG_CACHE:
        _PROG_CACHE[nreps] = _build_program(nreps)
    return _PROG_CACHE[nreps]


def _host_prep(x_cls, x_patch, q_w, k_w, v_w, temp, proj_w, proj_b):
    scale = 1.0 / math.sqrt(HD)
    tvec = np.repeat(temp.reshape(H).astype(np.float64), HD)
    q_ws = q_w.astype(np.float64) * (scale * tvec)[:, None]
    k64 = k_w.astype(np.float64)
    v64 = v_w.astype(np.float64)
    p64 = proj_w.astype(np.float64)
    g = np.empty((H, C, C), dtype=np.float64)
    w = np.empty((H, C, C), dtype=np.float64)
    for h in range(H):
        sl = slice(h * HD, (h + 1) * HD)
        g[h] = q_ws[sl, :].T @ k64[sl, :]          # [cin'(K), cin(M)]
        w[h] = (p64[:, sl] @ v64[sl, :]).T         # W_h.T = [cin(K), co(M)]
    g_bf = np.ascontiguousarray((g * ALPHA).astype(ml_dtypes.bfloat16))
    w_bf = np.ascontiguousarray(w.astype(ml_dtypes.bfloat16))
    pb = np.ascontiguousarray(proj_b.reshape(C, 1).astype(np.float32))
    return g_bf, w_bf, pb


def _make_in_maps(x_cls, x_patch, g_bf, w_bf, pb):
    xp_full = x_patch.reshape(B, C, KT)
    xpb = xp_full.astype(ml_dtypes.float8_e4m3)              # [B, C, KT]
    xpt = np.ascontiguousarray(
        xp_full.astype(ml_dtypes.bfloat16).transpose(0, 2, 1))  # [B, KT, C]
    xc = np.ascontiguousarray(x_cls.reshape(B, C, V).astype(np.float32))
    in_maps = []
    for c in range(NCORES):
        bs = slice(c * BLOC, (c + 1) * BLOC)
        in_maps.append({
            "xc": xc[bs],
            "xpb": np.ascontiguousarray(xpb[bs]),
            "xpt": xpt[bs],
            "g": g_bf, "w": w_bf, "pb": pb,
        })
    return in_maps


def kernel(x_cls, x_patch, q_w, k_w, v_w, temp, proj_w, proj_b):
    g_bf, w_bf, pb = _host_prep(
        x_cls, x_patch, q_w, k_w, v_w, temp, proj_w, proj_b)
    nc = _get_program()
    in_maps = _make_in_maps(x_cls, x_patch, g_bf, w_bf, pb)
    res = bass_utils.run_bass_kernel_spmd(
        nc, in_maps, core_ids=list(range(NCORES)))
    out = np.concatenate([res.results[c]["y"] for c in range(NCORES)], axis=0)
    return out.reshape(B, C, 1, V).astype(np.float32)


# revision 31
# speedup vs baseline: 1.0497x; 1.0497x over previous
"""Trainium2 Bass kernel for ClassAttentionTSSA.

Reference computation (B=64, C=256, T=64, V=25, h=8, hd=32):
    xc = x_cls  as (B, V, C) tokens;  xp = x_patch as (B, T*V, C) tokens
    q = xc @ q_w.T ; k = xp @ k_w.T ; v = xp @ v_w.T   (per-head split hd=32)
    S = (q @ k.T) * scale * temp_h ; A = softmax(S) ; o = A @ v
    y = concat_heads(o) @ proj_w.T + proj_b  -> (B, C, 1, V)

Weight-only reassociations (exact up to fp reordering):
    S_h = xc @ G_h @ xp.T    with G_h = (q_w*scale*temp)_h.T @ k_w_h  (C x C)
    y   = sum_h (A_h @ xp) @ W_h.T + b   with W_h = proj_w[:,h] @ v_w[h,:]
so q/k/v are never materialized.  Channels live on partitions, tokens on
the free dim.  All inputs are DMA'd into SBUF ONCE at program start
(everything fits; x_patch is shipped in both layouts: [cin,kt] as fp8e4
for the S matmuls and [kt,cin] as bf16 for the ctx matmuls), so the
steady-state rep loop is pure compute + one output DMA.

Per-core pipeline per rep (8 batches, software-pipelined per step s:
recip(s-1) | S+exp(s) | ctx(s-1) | Z(s)):
    B) S^T chunk-group matmuls: ONE fp8e4 DoubleRow matmul per 128-kt
       chunk (K=256 contracted in one instruction at 0.5 cyc/col).
       qkT is pre-scaled by ALPHA=128 (folded into G on the host) so
       fp8e4 sees ~unit-variance values; the Exp's free scale undoes it.
       Softmax numerator: ONE ACT Exp per 4-chunk PSUM group, straight
       PSUM->SBUF bf16.  No max-subtraction (|S| < 1 for this data).
    C) Z: the 12 full attn chunks fold pairwise on DVE (bf16 level 1,
       f32 above -- one rounding total), dependent only on Exps 1-3;
       then TWO all-ones matmuls on PE (folded sum + chunk 12) that
       partition-reduce AND broadcast Z into PSUM in one accumulation
       group, then DVE reciprocal_approx_fast.
    D) ctxT accumulation matmuls (bf16), normalized by 1/Z on DVE.
    E) y^T = sum_h W_h^T @ ctxT + pb, DMA out.

Accuracy: rel-to-absmax ~9.6e-3 (gate 2e-2); the fp8 S-side is the
dominant term (bf16-everything measures ~3.4e-3); validated in numpy
before committing.  fp8 on the ctx side fails (4.1e-2) -- do not.

Perf notes for this axon-tunneled environment (all MEASURED here):
  - HW exec: 48.3us/rep (baseline inherited: 158.8us; same-code
    run-to-run drift up to ~15% was observed across the session).
    PE is instruction-ISSUE-limited (~110ns/matmul): per batch
    13 S + 2 Z + 26 ctx matmuls + 32 for stage E per rep.
  - For_i back-edge all-engine barrier ~4.1us/iter.  Putting MULTIPLE
    rep bodies inside one For_i iteration REGRESSES badly (+19us per
    extra body: 2-body 69us/rep, 4-body 80us/rep) -- do not unroll.
    For_i(staggered_reset=True) hard-crashes the device (NRT 101).
  - ACT table funcs cost the same as Copy (~670ns/800col + ~390ns
    fixed): Exp==Square==Copy.  The previous session's "Exp ~28us"
    note was wrong; exp-via-DVE-quartic is unnecessary.
  - DVE: TS bf16 2x (~428ns/800col), TT bf16 2x (~363ns/800col), STT
    and TensorReduce have NO fast modes (1x); reciprocal is 8cyc/elem
    (use reciprocal_approx_fast, ~5x faster, 18-bit); DVE ops may read
    at most ONE non-scalar input from PSUM.
  - Cross-engine semaphore latency is small (~0.1-0.2us), but work
    placed AFTER the last Exp of a batch lands on the per-step
    critical path: keep the post-Exp4 tail to ONE Z matmul (chunk 12);
    folds that depend on Exp4 cost ~+1us/step (measured 59us).
  - gpsimd compute ops ~16us each (avoid; SWDGE cast-DMA is fine).

Sharding: data-parallel over batch, 8 batches per NeuronCore, 8 cores.
"""

import math
import sys

sys.path.insert(0, "/opt/trn_rl_repo")

import numpy as np
import ml_dtypes

import concourse.bacc as bacc
import concourse.mybir as mybir
import concourse.tile as tile
from concourse import bass_utils

B, C, T, V = 64, 256, 64, 25
H, HD = 8, 32
KT = T * V            # 1600 key tokens
NCORES = 8
BLOC = B // NCORES    # 8 batches per core
R = H * V             # 200 packed (head, query) columns per batch
CK = C // 128         # 2 channel chunks

F32 = mybir.dt.float32
BF16 = mybir.dt.bfloat16
FP8 = mybir.dt.float8e4
ALPHA = 128.0         # qkT pre-scale so fp8e4 uses its dynamic range;
                      # folded into G on the host, undone by the Exp scale

KT_CHUNKS = [128] * (KT // 128) + ([KT % 128] if KT % 128 else [])
NM = len(KT_CHUNKS)   # 13
NM_FULL = KT // 128   # 12
TILE_CH = 4           # S^T chunks per 2-bank psum tile (2 per bank)

_PROG_CACHE = {}
_SIM_UNROLL = False   # sim tooling sets True: plain body instead of For_i
                      # (TimelineSim cannot resolve register-mode branches)


def _build_program(nreps: int = 1):
    """Build + compile the per-core Bass program (same program on all cores)."""
    from contextlib import ExitStack, nullcontext

    MULT, ADD = mybir.AluOpType.mult, mybir.AluOpType.add
    AF = mybir.ActivationFunctionType

    nc = bacc.Bacc("TRN2", target_bir_lowering=False, debug=False)

    xc_d = nc.dram_tensor("xc", [BLOC, C, V], F32, kind="ExternalInput")
    xpb_d = nc.dram_tensor("xpb", [BLOC, C, KT], FP8, kind="ExternalInput")
    xpt_d = nc.dram_tensor("xpt", [BLOC, KT, C], BF16, kind="ExternalInput")
    g_d = nc.dram_tensor("g", [H, C, C], BF16, kind="ExternalInput")
    w_d = nc.dram_tensor("w", [H, C, C], BF16, kind="ExternalInput")
    pb_d = nc.dram_tensor("pb", [C, 1], F32, kind="ExternalInput")
    y_d = nc.dram_tensor("y", [BLOC, C, V], F32, kind="ExternalOutput")

    with tile.TileContext(nc) as tc, ExitStack() as es:
        wpool = es.enter_context(tc.tile_pool(name="weights", bufs=1))
        xpool = es.enter_context(tc.tile_pool(name="xdata", bufs=1))
        attn_pool = es.enter_context(tc.tile_pool(name="attn", bufs=1))
        zpool = es.enter_context(tc.tile_pool(name="zdata", bufs=1))
        ysb_pool = es.enter_context(tc.tile_pool(name="ysb", bufs=2))

        # ---- persistent weights / activations (one DMA each) ----
        g_sb = wpool.tile([128, H * CK * C], BF16, tag="g")
        nc.sync.dma_start(
            g_sb[:].rearrange("p (h kc j) -> p h kc j", h=H, kc=CK),
            g_d.ap().rearrange("h (kc p) j -> p h kc j", kc=CK),
        )
        w_sb = wpool.tile([128, H * CK * C], BF16, tag="w")
        nc.sync.dma_start(
            w_sb[:].rearrange("p (h kc j) -> p h kc j", h=H, kc=CK),
            w_d.ap().rearrange("h (kc p) j -> p h kc j", kc=CK),
        )
        pb_sb = wpool.tile([128, CK], F32, tag="pb")
        nc.sync.dma_start(
            pb_sb[:], pb_d.ap().rearrange("(kc p) one -> p (kc one)", kc=CK))
        xcT = wpool.tile([128, CK * BLOC * V], BF16, tag="xc")
        for kc in range(CK):
            nc.gpsimd.dma_start(  # SWDGE: casts f32 -> bf16 in flight
                xcT[:, kc * BLOC * V:(kc + 1) * BLOC * V].rearrange(
                    "p (b v) -> p b v", b=BLOC),
                xc_d.ap()[:, kc * 128:(kc + 1) * 128, :].rearrange(
                    "b p v -> p b v"),
            )
        ones_sb = wpool.tile([128, 128], BF16, tag="ones")
        nc.vector.memset(ones_sb[:], 1.0)
        ones_f32 = wpool.tile([128, 128], F32, tag="ones32")
        nc.vector.memset(ones_f32[:], 1.0)
        zero_bias = wpool.tile([128, 1], F32, tag="zb")
        nc.vector.memset(zero_bias[:], 0.0)

        # qkT cols: (kc | b, h, qi)  b-major: S^T rhs slices contiguous
        qkT = wpool.tile([128, CK * BLOC * R], FP8, tag="qkT")
        # ctxT cols: (kc | h, b, qi) h-major: y rhs slices contiguous
        ctxT = wpool.tile([128, CK * BLOC * R], BF16, tag="ctxT")

        # per-batch persistent input tiles + attn tiles (loaded ONCE)
        xpb_sb = [xpool.tile([128, CK * KT], FP8, tag=f"xpb{b}",
                             name=f"xpb{b}") for b in range(BLOC)]
        xpt_sb = [xpool.tile([128, NM * C], BF16, tag=f"xpt{b}",
                             name=f"xpt{b}") for b in range(BLOC)]
        attn_sb = [attn_pool.tile([128, NM * R], BF16, tag=f"attn{b}",
                              name=f"attn{b}") for b in range(BLOC)]
        for b in range(BLOC):
            nc.sync.dma_start(
                xpb_sb[b][:].rearrange("p (kc j) -> p kc j", kc=CK),
                xpb_d.ap()[b].rearrange("(kc p) j -> p kc j", kc=CK),
            )
            nc.sync.dma_start(
                xpt_sb[b][:, 0:NM_FULL * C].rearrange(
                    "p (m j) -> p m j", m=NM_FULL),
                xpt_d.ap()[b, 0:NM_FULL * 128, :].rearrange(
                    "(m p) j -> p m j", p=128),
            )
            nc.sync.dma_start(
                xpt_sb[b][0:KT - NM_FULL * 128, NM_FULL * C:NM * C],
                xpt_d.ap()[b, NM_FULL * 128:KT, :],
            )
            # rows 64:128 of the 64-row last chunk are never written by
            # the exp -- zero them once so the Z-reduce sees zeros there.
            nc.vector.memset(attn_sb[b][KT % 128:128, NM_FULL * R:NM * R], 0.0)

        recip = zpool.tile([128, BLOC * R], F32, tag="recip")
        # pairwise folds of attn chunks 0-11 on DVE (bf16 level 1 only --
        # later levels are f32 so Z keeps one rounding); depends only on
        # Exps 1-3, so the post-Exp4 critical tail stays 1 matmul.
        zs_sb = [zpool.tile([128, 6 * R], BF16, tag=f"zs{b}",
                            name=f"zs{b}") for b in range(BLOC)]
        zs2_sb = [zpool.tile([128, 3 * R], F32, tag=f"zs2{b}",
                             name=f"zs2{b}") for b in range(BLOC)]
        za_sb = [zpool.tile([128, R], F32, tag=f"za{b}",
                            name=f"za{b}") for b in range(BLOC)]
        zsum_sb = [zpool.tile([128, R], F32, tag=f"zsum{b}",
                              name=f"zsum{b}") for b in range(BLOC)]

        # ---- phase 1: qkT[cin, (b,h,qi)] = G_h^T @ xcT ----
        with tc.tile_pool(name="ps_qk", bufs=2, space="PSUM") as ps_qk:
            for mc in range(CK):
                for hg in range(2):          # head groups of 4
                    pq = ps_qk.tile([128, 4 * 512], F32, tag="pq")
                    for i in range(4):
                        h = hg * 4 + i
                        for kc in range(CK):
                            nc.tensor.matmul(
                                pq[:, i * 512:i * 512 + BLOC * V],
                                g_sb[:, (h * CK + kc) * C + mc * 128:
                                     (h * CK + kc) * C + mc * 128 + 128],
                                xcT[:, kc * BLOC * V:(kc + 1) * BLOC * V],
                                start=(kc == 0), stop=(kc == CK - 1),
                            )
                    nc.vector.tensor_copy(
                        qkT[:, mc * BLOC * R:(mc + 1) * BLOC * R]
                        .rearrange("p (b h q) -> p b h q", b=BLOC, h=H)
                        [:, :, hg * 4:(hg + 1) * 4, :],
                        pq[:].rearrange("p (i n) -> p i n", i=4)
                        [:, :, 0:BLOC * V]
                        .rearrange("p i (b q) -> p b i q", q=V),
                    )

        ps_st = es.enter_context(
            tc.tile_pool(name="ps_st", bufs=2, space="PSUM"))
        ps_z = es.enter_context(
            tc.tile_pool(name="ps_z", bufs=2, space="PSUM"))
        ps_acc = es.enter_context(
            tc.tile_pool(name="ps_acc", bufs=2, space="PSUM"))

        with (nullcontext(0) if _SIM_UNROLL else tc.For_i(0, nreps)) as _rep:
            # ---- stages B/C/D: per-batch software pipeline ----
            # step s emits S^T+exp for batch s, then Z/recip/ctx for
            # batch s-1, so PE's ctx matmuls overlap the ACT-limited
            # stretch of the next batch's exp.
            pz_live = {}

            def emit_front(b):
                # S^T chunk-group matmuls + one ACT Exp per group
                m = 0
                while m < NM:
                    gsz = min(TILE_CH, NM - m)
                    if KT_CHUNKS[m + gsz - 1] != KT_CHUNKS[m]:
                        gsz -= 1
                    rows = KT_CHUNKS[m]
                    st = ps_st.tile([128, 2 * 512], F32, tag="st")
                    for i in range(gsz):
                        # fp8 DoubleRow: contracts both 128-channel halves
                        # (K=256) in ONE matmul at half cycles/col
                        nc.tensor.matmul(
                            st[0:rows, i * 256:i * 256 + R],
                            xpb_sb[b][:].rearrange(
                                "p (kc j) -> p kc j", kc=CK)
                            [:, :, (m + i) * 128:
                             (m + i) * 128 + KT_CHUNKS[m + i]],
                            qkT[:].rearrange(
                                "p (kc n) -> p kc n", kc=CK)
                            [:, :, b * R:(b + 1) * R],
                            start=True, stop=True,
                            perf_mode=mybir.MatmulPerfMode.DoubleRow,
                        )
                    x = st[0:rows, :].rearrange(
                        "p (g n) -> p g n", n=256)[:, 0:gsz, 0:R]
                    # softmax numerator straight from PSUM on ACT
                    nc.scalar.activation(
                        attn_sb[b][0:rows, m * R:(m + gsz) * R].rearrange(
                            "p (g n) -> p g n", g=gsz),
                        x, AF.Exp, bias=zero_bias[0:rows, :],
                        scale=1.0 / ALPHA)
                    m += gsz
                    if m == 12:
                        nc.vector.tensor_add(
                            zs_sb[b][:], attn_sb[b][:, 0:6 * R],
                            attn_sb[b][:, 6 * R:12 * R])
                        nc.vector.tensor_add(
                            zs2_sb[b][:], zs_sb[b][:, 0:3 * R],
                            zs_sb[b][:, 3 * R:6 * R])
                        nc.vector.tensor_add(
                            za_sb[b][:], zs2_sb[b][:, 0:R],
                            zs2_sb[b][:, R:2 * R])
                        nc.vector.tensor_add(
                            zsum_sb[b][:], za_sb[b][:],
                            zs2_sb[b][:, 2 * R:3 * R])

            def emit_pz(b):
                # Z on PE: one all-ones matmul per exp'd chunk (contracts
                # over kt partitions AND broadcasts over the 128 out
                # partitions in one op).  Emitted late in the step so the
                # Exp(b) -> pz(b) wait has slack behind ctx(b-1).
                pz = ps_z.tile([128, 512], F32, tag="pz")
                pz_live[b] = pz
                nc.tensor.matmul(
                    pz[:, 0:R], ones_f32[:], zsum_sb[b][:],
                    start=True, stop=False,
                )
                nc.tensor.matmul(
                    pz[:, 0:R], ones_sb[:],
                    attn_sb[b][:, 12 * R:13 * R],
                    start=False, stop=True,
                )

            def emit_recip(b):
                pz = pz_live.pop(b)
                nc.vector.reciprocal_approx_fast(
                    recip[:, b * R:(b + 1) * R], pz[:, 0:R])

            def emit_ctx(b):
                # ctxT[cin, (h,qi)] = sum_kt xp_kt^T @ A^T, * 1/Z
                for mc in range(CK):
                    pc = ps_acc.tile([128, 512], F32, tag="pc")
                    for m in range(NM):
                        nc.tensor.matmul(
                            pc[:, 0:R],
                            xpt_sb[b][0:KT_CHUNKS[m], m * C + mc * 128:
                                      m * C + mc * 128 + 128],
                            attn_sb[b][0:KT_CHUNKS[m], m * R:(m + 1) * R],
                            start=(m == 0), stop=(m == NM - 1),
                        )
                    nc.vector.tensor_mul(
                        ctxT[:, mc * BLOC * R:(mc + 1) * BLOC * R].rearrange(
                            "p (h b q) -> p h b q", h=H, b=BLOC)[:, :, b, :],
                        pc[:, 0:R].rearrange("p (h q) -> p h q", h=H),
                        recip[:, b * R:(b + 1) * R].rearrange(
                            "p (h q) -> p h q", h=H),
                    )

            def rep_body():
                for s in range(BLOC + 1):
                    if s >= 1:
                        emit_recip(s - 1)
                    if s < BLOC:
                        emit_front(s)
                    if s >= 1:
                        emit_ctx(s - 1)
                    if s < BLOC:
                        emit_pz(s)

                # ---- stage E: y^T = sum_h W_h^T @ ctxT + pb ----
                for mc in range(CK):
                    py = ps_acc.tile([128, 512], F32, tag="pc")
                    idx = 0
                    for h in range(H):
                        for kc in range(CK):
                            nc.tensor.matmul(
                                py[:, 0:BLOC * V],
                                w_sb[:, (h * CK + kc) * C + mc * 128:
                                     (h * CK + kc) * C + mc * 128 + 128],
                                ctxT[:, kc * BLOC * R + h * BLOC * V:
                                     kc * BLOC * R + (h + 1) * BLOC * V],
                                start=(idx == 0), stop=(idx == 2 * H - 1),
                            )
                            idx += 1
                    ysb = ysb_pool.tile([128, BLOC * V], F32, tag="ysb")
                    nc.vector.tensor_scalar_add(
                        ysb[:], py[:, 0:BLOC * V], pb_sb[:, mc:mc + 1])
                    nc.sync.dma_start(
                        y_d.ap()[:, mc * 128:(mc + 1) * 128, :].rearrange(
                            "b p v -> p b v"),
                        ysb[:].rearrange("p (b v) -> p b v", b=BLOC),
                    )

            if _SIM_UNROLL:
                for _ in range(nreps - 1):
                    rep_body()
            rep_body()

    nc.compile()
    return nc


def _get_program(nreps: int = 1):
    if nreps not in _PROG_CACHE:
        _PROG_CACHE[nreps] = _build_program(nreps)
    return _PROG_CACHE[nreps]


def _host_prep(x_cls, x_patch, q_w, k_w, v_w, temp, proj_w, proj_b):
    scale = 1.0 / math.sqrt(HD)
    tvec = np.repeat(temp.reshape(H).astype(np.float64), HD)
    q_ws = q_w.astype(np.float64) * (scale * tvec)[:, None]
    k64 = k_w.astype(np.float64)
    v64 = v_w.astype(np.float64)
    p64 = proj_w.astype(np.float64)
    g = np.empty((H, C, C), dtype=np.float64)
    w = np.empty((H, C, C), dtype=np.float64)
    for h in range(H):
        sl = slice(h * HD, (h + 1) * HD)
        g[h] = q_ws[sl, :].T @ k64[sl, :]          # [cin'(K), cin(M)]
        w[h] = (p64[:, sl] @ v64[sl, :]).T         # W_h.T = [cin(K), co(M)]
    g_bf = np.ascontiguousarray((g * ALPHA).astype(ml_dtypes.bfloat16))
    w_bf = np.ascontiguousarray(w.astype(ml_dtypes.bfloat16))
    pb = np.ascontiguousarray(proj_b.reshape(C, 1).astype(np.float32))
    return g_bf, w_bf, pb


def _make_in_maps(x_cls, x_patch, g_bf, w_bf, pb):
    xp_full = x_patch.reshape(B, C, KT)
    xpb = xp_full.astype(ml_dtypes.float8_e4m3)              # [B, C, KT]
    xpt = np.ascontiguousarray(
        xp_full.astype(ml_dtypes.bfloat16).transpose(0, 2, 1))  # [B, KT, C]
    xc = np.ascontiguousarray(x_cls.reshape(B, C, V).astype(np.float32))
    in_maps = []
    for c in range(NCORES):
        bs = slice(c * BLOC, (c + 1) * BLOC)
        in_maps.append({
            "xc": xc[bs],
            "xpb": np.ascontiguousarray(xpb[bs]),
            "xpt": xpt[bs],
            "g": g_bf, "w": w_bf, "pb": pb,
        })
    return in_maps


def kernel(x_cls, x_patch, q_w, k_w, v_w, temp, proj_w, proj_b):
    g_bf, w_bf, pb = _host_prep(
        x_cls, x_patch, q_w, k_w, v_w, temp, proj_w, proj_b)
    nc = _get_program()
    in_maps = _make_in_maps(x_cls, x_patch, g_bf, w_bf, pb)
    res = bass_utils.run_bass_kernel_spmd(
        nc, in_maps, core_ids=list(range(NCORES)))
    out = np.concatenate([res.results[c]["y"] for c in range(NCORES)], axis=0)
    return out.reshape(B, C, 1, V).astype(np.float32)
